# revision 26
# baseline (speedup 1.0000x reference)
"""Trainium2 kernel for DigitConvolutionalModel.

Model: x(B,784) -> reshape(28,28) -> conv3x3 'VALID' (cross-correlation)
       -> flatten(676) -> Linear(676,256)+ReLU -> Linear(256,10).

The conv is linear, so it folds into the first Linear:
    feat = x @ Wc          (Wc: 784x676 sparse conv matrix)
    h    = relu(feat @ w1 + b1) = relu(x @ (Wc @ w1) + b1)
Device work is two GEMMs per batch block:
    H^T = relu(W_eff^T-tiles . x^T + b1);  out^T = w2^T . H

Sharding: pure data parallel over 8 cores (8192 rows each). Host
pre-packs each shard as x^T in bf16 (contraction on SBUF partitions);
all of x stays resident in SBUF (115 KB/partition), so the batch is
loaded with just 8 DMAs of ramped size and never recycled.

Perf notes (vs the 82us baseline):
  - GEMM2's two K=128 halves run column-tiled (tile_position (0,0) and
    (0,32)) in one PE slot; the halves are summed on the DVE into SBUF
    and the +b2 moves to the host unshard, cutting a PE slot per block.
  - ~10 warm-up matmuls on a memset tile run while the first x chunk
    loads, so the HAM clock-gate reaches K=8/8 before real work.
  - x arrives as one 3D-AP DMA per chunk (128..2048 cols) on the sync
    ring; weights ride the ACT ring; stores ride the GpSimd ring.
"""

import os
from contextlib import ExitStack

import numpy as np
import ml_dtypes

import concourse.bass as bass
import concourse.tile as tile
from concourse import bacc, mybir
from concourse.bass_utils import run_bass_kernel_spmd

N_CORES = 8
B = 65536
B_SHARD = B // N_CORES  # 8192
K = 784                 # contraction dim (pixels)
KT = 112                # k-tile partition size (7 * 112 = 784)
NKT = K // KT
CH = 256                # hidden channels
MT = 128                # m-tile (output channels per matmul)
NMT = CH // MT
OUT_CH = 10
OUT_PAD = 16            # padded output channels per GEMM2 half
CHUNKS = [256, 256, 512, 1024, 2048, 2048, 2048]  # x DMA chunk sizes (cols)
N_FILL_CHUNKS = 4       # chunks whose matmuls get dummy-fill interleave
FILL = 4                # dummy matmuls inserted after each ramp matmul
SUB = 512               # max matmul moving free dim / PSUM bank
N_WARMUP = 50           # dummy matmuls to warm the HAM clock gate
BF16 = mybir.dt.bfloat16
F32 = mybir.dt.float32

_CACHE: dict = {}


def _build(b_shard: int):
    assert sum(CHUNKS) == b_shard
    nc = bacc.Bacc(
        "TRN2",
        target_bir_lowering=False,
        debug=False,
        num_devices=N_CORES,
    )
    # x^T packed [KT, NKT, b_shard]: xp[p, t, col] = x[col, t*KT+p]
    xT = nc.dram_tensor("xT", [KT, NKT, b_shard], BF16, kind="ExternalInput")
    # GEMM1 weight tiles packed side by side, m-major: [112, (m*NKT+t)*MT + j]
    wta = nc.dram_tensor("wta", [KT, NKT * NMT * MT], BF16, kind="ExternalInput")
    b1a = nc.dram_tensor("b1a", [MT, NMT], F32, kind="ExternalInput")
    # w2 halves side by side: [128, m*OUT_PAD + j] = w2[m*128 + p, j]
    w2a = nc.dram_tensor("w2a", [MT, NMT * OUT_PAD], BF16, kind="ExternalInput")
    outT = nc.dram_tensor("outT", [OUT_PAD, b_shard], F32, kind="ExternalOutput")

    relu = mybir.ActivationFunctionType.Relu

    with tile.TileContext(nc) as tc, ExitStack() as ctx:
        xpool = ctx.enter_context(tc.tile_pool(name="xin", bufs=1))
        const = ctx.enter_context(tc.tile_pool(name="const", bufs=1))
        # GEMM2 constants in their own pool: sharing the bufs=1 const pool
        # with the GEMM1 weights trips a scheduler slot-wait deadlock.
        const2 = ctx.enter_context(tc.tile_pool(name="const2", bufs=1))
        dpool = ctx.enter_context(tc.tile_pool(name="dummy", bufs=1))
        hpool = ctx.enter_context(tc.tile_pool(name="h", bufs=3))
        opool = ctx.enter_context(tc.tile_pool(name="out", bufs=2))
        hps = ctx.enter_context(
            tc.tile_pool(name="hps", bufs=2, space=bass.MemorySpace.PSUM)
        )
        ops = ctx.enter_context(
            tc.tile_pool(name="ops", bufs=3, space=bass.MemorySpace.PSUM)
        )
        dps = ctx.enter_context(
            tc.tile_pool(name="dps", bufs=1, space=bass.MemorySpace.PSUM)
        )

        # --- x loads first: they are the critical path.  Every chunk is
        # split per k-tile so a block's matmul t only waits for its own
        # piece, not the whole chunk (whole-chunk waits stall the PE long
        # enough to re-throttle the HAM clock gate).  Chunk sizes ramp so
        # descriptor runs grow 1KB -> 4KB (DMA rate ~200 -> ~305 GB/s)
        # while the PE is still cold and cheap to feed. ---
        # x pieces alternate between the sync (HWDGE) and gpsimd (SWDGE)
        # rings: the ~280 GB/s ceiling is per-ring descriptor service, two
        # rings get closer to the ~358 GB/s HBM share.
        xt_chunks = []
        coff = 0
        piece = 0
        for c, csz in enumerate(CHUNKS):
            xtile = xpool.tile([KT, NKT * csz], BF16, tag=f"x{c}", name=f"x{c}")
            for t in range(NKT):
                ring = nc.sync if piece % 2 == 0 else nc.gpsimd
                ring.dma_start(
                    xtile[:, t * csz:(t + 1) * csz],
                    xT[:, t, coff:coff + csz],
                )
                piece += 1
            xt_chunks.append(xtile)
            coff += csz

        # --- resident weights/biases on the ACT ring. The m=0/t=0 slice
        # loads alone first (29KB) so the first matmul isn't gated on the
        # full 400KB weight load. ---
        HW = NKT * MT
        wt_m = []
        for m in range(NMT):
            wtile = const.tile([KT, HW], BF16, tag=f"wta{m}", name=f"wt_m{m}")
            if m == 0:
                nc.scalar.dma_start(wtile[:, 0:MT], wta[:, 0:MT])
                nc.scalar.dma_start(wtile[:, MT:HW], wta[:, MT:HW])
            else:
                nc.scalar.dma_start(wtile[:], wta[:, m * HW:(m + 1) * HW])
            wt_m.append(wtile)
        b1_all = const.tile([MT, NMT], F32, tag="b1a")
        nc.scalar.dma_start(b1_all[:], b1a[:, :])
        w2_all = const2.tile([MT, NMT * OUT_PAD], BF16, tag="w2a")
        nc.scalar.dma_start(w2_all[:], w2a[:, :])

        # --- HAM warm-up: dummy matmuls on a zeroed tile keep the PE
        # busy while the first x chunk loads, so the clock gate is at
        # K=8/8 by the time real matmuls issue. ---
        dummy = dpool.tile([MT, MT], BF16, tag="dm")
        nc.vector.memset(dummy[:], 0.0)
        dm_ps = dps.tile([MT, MT], F32, tag="dps")

        def dummy_mm(n=1):
            for _ in range(n):
                nc.tensor.matmul(
                    dm_ps[0:32, 0:32], dummy[0:32, 0:32], dummy[0:32, 0:32],
                    start=True, stop=True,
                )

        dummy_mm(N_WARMUP)

        def w_sb(t, m):
            return wt_m[m][:, t * MT:(t + 1) * MT]

        # GEMM2 + store for one finished block, one block behind GEMM1
        # (software pipeline) so the PE never stalls on the ACT relu.
        def flush_gemm2(pend):
            hb, j0, bsz = pend
            po = ops.tile([OUT_PAD, SUB], F32, tag="po", name="po")
            for m in range(NMT):
                nc.tensor.matmul(
                    po[:, :bsz],
                    w2_all[:, m * OUT_PAD:(m + 1) * OUT_PAD],
                    hb[m][:, :bsz],
                    start=(m == 0),
                    stop=(m == NMT - 1),
                )
            # DMA cannot read PSUM: stage through SBUF on the (idle) DVE.
            ob = opool.tile([OUT_PAD, SUB], F32, tag="ob", name="ob")
            nc.vector.tensor_copy(ob[:, :bsz], po[:, :bsz])
            nc.scalar.dma_start(outT[:, j0:j0 + bsz], ob[:, :bsz])

        # --- main loop over batch blocks (<=512 cols each) ---
        pending = None
        coff = 0
        for c, csz in enumerate(CHUNKS):
            xtile = xt_chunks[c]
            for s in range(max(1, csz // SUB)):
                bsz = min(csz, SUB)
                hb = []
                fill = FILL if c < N_FILL_CHUNKS else 0
                for m in range(NMT):
                    ps = hps.tile([MT, SUB], F32, tag=f"ps{m}")
                    for t in range(NKT):
                        nc.tensor.matmul(
                            ps[:, :bsz],
                            w_sb(t, m),
                            xtile[:, t * csz + s * SUB:t * csz + s * SUB + bsz],
                            start=(t == 0),
                            stop=(t == NKT - 1),
                        )
                        # ramp phase: tiny dummy matmuls plug DMA-supply
                        # holes so the PE stays busy and the HAM clock
                        # gate never re-throttles.
                        dummy_mm(fill)
                    h = hpool.tile([MT, SUB], BF16, tag=f"h{m}")
                    nc.scalar.activation(
                        h[:, :bsz], ps[:, :bsz], relu, bias=b1_all[:, m:m + 1]
                    )
                    hb.append(h)
                    if m == 0 and pending is not None:
                        flush_gemm2(pending)
                        pending = None
                pending = (hb, coff + s * SUB, bsz)
            coff += csz
        flush_gemm2(pending)

    nc.compile()
    return nc


def _get_nc(b_shard: int = B_SHARD):
    if b_shard not in _CACHE:
        _CACHE[b_shard] = _build(b_shard)
    return _CACHE[b_shard]


def _host_prep(x, w_conv, w1, b1, w2, b2, b_shard=B_SHARD):
    """Fold conv into w1, pack weights, and lay out per-core inputs."""
    bf16 = ml_dtypes.bfloat16
    # Conv matrix Wc[784, 676]: feat[:, oi*26+oj] = sum_{di,dj} x[:, (oi+di)*28+(oj+dj)] * w_conv[di,dj]
    w_conv = np.asarray(w_conv, np.float64)
    oi = np.arange(26)
    oj = np.arange(26)
    wc = np.zeros((784, 676), np.float64)
    for di in range(3):
        for dj in range(3):
            src = ((oi[:, None] + di) * 28 + (oj[None, :] + dj)).ravel()
            dst = (oi[:, None] * 26 + oj[None, :]).ravel()
            wc[src, dst] += w_conv[di, dj]
    w_eff = (wc @ np.asarray(w1, np.float64)).astype(bf16)  # [784, 256]

    # wta[p, (m*NKT+t)*MT + j] = w_eff[t*KT+p, m*MT+j]  (m-major)
    wta = np.ascontiguousarray(
        w_eff.reshape(NKT, KT, NMT, MT).transpose(1, 2, 0, 3).reshape(KT, -1)
    )
    # b1a[p, m] = b1[m*MT+p]
    b1a = np.ascontiguousarray(
        np.asarray(b1, np.float32).reshape(NMT, MT).T
    )
    # w2a[p, m*OUT_PAD + j] = w2_padded[m*MT+p, j]
    w2p = np.zeros((CH, OUT_PAD), bf16)
    w2p[:, :OUT_CH] = np.asarray(w2).astype(bf16)
    w2a = np.ascontiguousarray(
        w2p.reshape(NMT, MT, OUT_PAD).transpose(1, 0, 2).reshape(MT, -1)
    )

    x_bf = np.asarray(x).astype(bf16)  # [B, 784]
    in_maps = []
    for c in range(N_CORES):
        shard = x_bf[c * b_shard:(c + 1) * b_shard]
        # xT[p, t, col] = shard[col, t*KT+p]
        xp = np.ascontiguousarray(
            shard.T.reshape(NKT, KT, b_shard).transpose(1, 0, 2)
        )
        in_maps.append({"xT": xp, "wta": wta, "b1a": b1a, "w2a": w2a})
    return in_maps


LAST_RESULT = None  # BassKernelResults of the most recent run (for test harness)


def kernel(x, w_conv, w1, b1, w2, b2):
    global LAST_RESULT
    nc = _get_nc()
    in_maps = _host_prep(x, w_conv, w1, b1, w2, b2)
    trace = bool(int(os.environ.get("KERNEL_TRACE", "0")))
    res = run_bass_kernel_spmd(
        nc, in_maps, list(range(N_CORES)), trace=trace,
        tmpdir=os.environ.get("KERNEL_TMPDIR") or None,
    )
    LAST_RESULT = res
    b2f = np.asarray(b2, np.float32)
    out = np.empty((B, OUT_CH), np.float32)
    for c in range(N_CORES):
        out[c * B_SHARD:(c + 1) * B_SHARD] = (
            res.results[c]["outT"][:OUT_CH].T + b2f
        )
    return out


# revision 33
# speedup vs baseline: 1.2688x; 1.2688x over previous
"""Trainium2 kernel for DigitConvolutionalModel.

Model: x(B,784) -> reshape(28,28) -> conv3x3 'VALID' (cross-correlation)
       -> flatten(676) -> Linear(676,256)+ReLU -> Linear(256,10).

The conv is linear, so it folds into the first Linear:
    feat = x @ Wc          (Wc: 784x676 sparse conv matrix)
    h    = relu(feat @ w1 + b1) = relu(x @ (Wc @ w1) + b1)
Device work is two GEMMs per 512-col batch block:
    H^T = relu(W_eff^T-tiles . x^T + b1);  out^T = w2^T . H

Sharding: pure data parallel over 8 cores (8192 rows each).  Host packs
each shard as x^T bf16 with the contraction dim on SBUF partitions; all
of x stays resident in SBUF (115 KB/partition) and is loaded by 35
per-k-tile piece DMAs, so a matmul only ever waits for its own piece.

Scheduling (the part that matters on HW):
  - The batch is processed in 1024-col HALVES, k-major: for each k-tile
    t, all four open PSUM groups (2 blocks x 2 m-tiles) consume piece t
    back-to-back.  While the DMA supply is still ramping, each piece
    arrival unlocks ~0.9us of real PE work, keeping the PE duty high so
    the HAM clock gate (K=8/8 after ~3.4us busy, re-throttles to 1.2GHz
    on mostly-idle windows) never drops the clock.  Once pieces are
    resident the same loop runs back-to-back at 216ns/matmul.
  - ~38 warm-up matmuls on a memset tile cover the initial DMA latency.
  - GEMM2 for a half runs one half later (relu long since retired).
"""

import os
from contextlib import ExitStack

import numpy as np
import ml_dtypes

import concourse.bass as bass
import concourse.tile as tile
from concourse import bacc, mybir
from concourse.bass_utils import run_bass_kernel_spmd

N_CORES = 8
B = 65536
B_SHARD = B // N_CORES  # 8192
K = 784                 # contraction dim (pixels)
KT = 112                # k-tile partition size (7 * 112 = 784)
NKT = K // KT
CH = 256                # hidden channels
MT = 128                # m-tile (output channels per matmul)
NMT = CH // MT
OUT_CH = 10
OUT_PAD = 16            # padded output channels
CHUNKS = [2048, 2048, 2048, 1024, 1024]  # x DMA chunk sizes (cols)
SUB = 512               # matmul moving free dim / PSUM bank
HALF = 1024             # scheduling granularity: 2 blocks of SUB
N_WARMUP = 38           # 128-wide dummy matmuls to warm the HAM clock gate
BF16 = mybir.dt.bfloat16
F32 = mybir.dt.float32

_CACHE: dict = {}


def _build(b_shard: int):
    assert sum(CHUNKS) == b_shard
    nc = bacc.Bacc(
        "TRN2",
        target_bir_lowering=False,
        debug=False,
        num_devices=N_CORES,
    )
    # x^T packed [KT, NKT, b_shard]: xp[p, t, col] = x[col, t*KT+p]
    xT = nc.dram_tensor("xT", [KT, NKT, b_shard], BF16, kind="ExternalInput")
    # GEMM1 weight tiles packed side by side, m-major: [112, (m*NKT+t)*MT + j]
    wta = nc.dram_tensor("wta", [KT, NKT * NMT * MT], BF16, kind="ExternalInput")
    b1a = nc.dram_tensor("b1a", [MT, NMT], F32, kind="ExternalInput")
    # w2 halves side by side: [128, m*OUT_PAD + j] = w2[m*128 + p, j]
    w2a = nc.dram_tensor("w2a", [MT, NMT * OUT_PAD], BF16, kind="ExternalInput")
    outT = nc.dram_tensor("outT", [OUT_PAD, b_shard], F32, kind="ExternalOutput")

    relu = mybir.ActivationFunctionType.Relu

    with tile.TileContext(nc) as tc, ExitStack() as ctx:
        xpool = ctx.enter_context(tc.tile_pool(name="xin", bufs=1))
        const = ctx.enter_context(tc.tile_pool(name="const", bufs=1))
        # GEMM2 constants in their own pool: sharing the bufs=1 const pool
        # with the GEMM1 weights trips a scheduler slot-wait deadlock.
        const2 = ctx.enter_context(tc.tile_pool(name="const2", bufs=1))
        dpool = ctx.enter_context(tc.tile_pool(name="dummy", bufs=1))
        hpool = ctx.enter_context(tc.tile_pool(name="h", bufs=2))
        opool = ctx.enter_context(tc.tile_pool(name="out", bufs=2))
        hps = ctx.enter_context(
            tc.tile_pool(name="hps", bufs=1, space=bass.MemorySpace.PSUM)
        )
        ops = ctx.enter_context(
            tc.tile_pool(name="ops", bufs=2, space=bass.MemorySpace.PSUM)
        )
        dps = ctx.enter_context(
            tc.tile_pool(name="dps", bufs=1, space=bass.MemorySpace.PSUM)
        )

        # --- x piece loads first: they are the critical path.  All on the
        # sync HWDGE ring (SWDGE starves; pieces >=230KB so the ~600ns
        # per-issue cost stays above the wire rate). ---
        xt_chunks = []
        coff = 0
        for c, csz in enumerate(CHUNKS):
            xtile = xpool.tile([KT, NKT * csz], BF16, tag=f"x{c}", name=f"x{c}")
            for t in range(NKT):
                nc.sync.dma_start(
                    xtile[:, t * csz:(t + 1) * csz],
                    xT[:, t, coff:coff + csz],
                )
            xt_chunks.append(xtile)
            coff += csz

        # --- resident weights/biases on the ACT ring. The m=0/t=0 slice
        # loads alone first (29KB) so the first matmul isn't gated on the
        # full 400KB weight load. ---
        HW = NKT * MT
        wt_m = []
        for m in range(NMT):
            wtile = const.tile([KT, HW], BF16, tag=f"wta{m}", name=f"wt_m{m}")
            if m == 0:
                nc.scalar.dma_start(wtile[:, 0:MT], wta[:, 0:MT])
                nc.scalar.dma_start(wtile[:, MT:HW], wta[:, MT:HW])
            else:
                nc.scalar.dma_start(wtile[:], wta[:, m * HW:(m + 1) * HW])
            wt_m.append(wtile)
        b1_all = const.tile([MT, NMT], F32, tag="b1a")
        nc.scalar.dma_start(b1_all[:], b1a[:, :])
        w2_all = const2.tile([MT, NMT * OUT_PAD], BF16, tag="w2a")
        nc.scalar.dma_start(w2_all[:], w2a[:, :])

        # --- HAM warm-up: dummy matmuls on a zeroed tile keep the PE busy
        # while the first pieces and weights load. ---
        dummy = dpool.tile([MT, MT], BF16, tag="dm")
        nc.vector.memset(dummy[:], 0.0)
        dm_ps = dps.tile([MT, MT], F32, tag="dps")
        for _ in range(N_WARMUP):
            nc.tensor.matmul(dm_ps[:], dummy[:], dummy[:], start=True, stop=True)

        def w_sb(t, m):
            return wt_m[m][:, t * MT:(t + 1) * MT]

        # GEMM2 + store for one finished block (software-pipelined one
        # half behind GEMM1, so the relu that made h is long done).
        def flush_gemm2(pend):
            hb, j0 = pend
            po = ops.tile([OUT_PAD, SUB], F32, tag="po", name="po")
            for m in range(NMT):
                nc.tensor.matmul(
                    po[:],
                    w2_all[:, m * OUT_PAD:(m + 1) * OUT_PAD],
                    hb[m][:],
                    start=(m == 0),
                    stop=(m == NMT - 1),
                )
            # DMA cannot read PSUM: stage through SBUF on the (idle) DVE.
            ob = opool.tile([OUT_PAD, SUB], F32, tag="ob", name="ob")
            nc.vector.tensor_copy(ob[:], po[:])
            nc.scalar.dma_start(outT[:, j0:j0 + SUB], ob[:])

        # --- main loop: 1024-col halves, k-major over 4 PSUM groups ---
        halves = []  # (chunk_idx, col_off_in_chunk, abs_col_off)
        coff = 0
        for c, csz in enumerate(CHUNKS):
            for hoff in range(0, csz, HALF):
                halves.append((c, hoff, coff + hoff))
            coff += csz

        pending = []  # up to 2 blocks awaiting GEMM2
        for c, hoff, aoff in halves:
            xtile = xt_chunks[c]
            csz = CHUNKS[c]
            ps = [
                [
                    hps.tile([MT, SUB], F32, tag=f"ps{s}{m}", name=f"ps{s}{m}")
                    for m in range(NMT)
                ]
                for s in range(2)
            ]
            for t in range(NKT):
                for s in range(2):
                    for m in range(NMT):
                        off = t * csz + hoff + s * SUB
                        nc.tensor.matmul(
                            ps[s][m][:],
                            w_sb(t, m),
                            xtile[:, off:off + SUB],
                            start=(t == 0),
                            stop=(t == NKT - 1),
                        )
                # previous half's GEMM2 work, spread into this sweep
                if t in (1, 4) and pending:
                    flush_gemm2(pending.pop(0))
            new_pend = []
            for s in range(2):
                hb = []
                for m in range(NMT):
                    h = hpool.tile([MT, SUB], BF16, tag=f"h{s}{m}")
                    nc.scalar.activation(
                        h[:], ps[s][m][:], relu, bias=b1_all[:, m:m + 1]
                    )
                    hb.append(h)
                new_pend.append((hb, aoff + s * SUB))
            while pending:
                flush_gemm2(pending.pop(0))
            pending = new_pend
        while pending:
            flush_gemm2(pending.pop(0))

    nc.compile()
    return nc


def _get_nc(b_shard: int = B_SHARD):
    if b_shard not in _CACHE:
        _CACHE[b_shard] = _build(b_shard)
    return _CACHE[b_shard]


def _host_prep(x, w_conv, w1, b1, w2, b2, b_shard=B_SHARD):
    """Fold conv into w1, pack weights, and lay out per-core inputs."""
    bf16 = ml_dtypes.bfloat16
    # Conv matrix Wc[784, 676]: feat[:, oi*26+oj] = sum_{di,dj} x[:, (oi+di)*28+(oj+dj)] * w_conv[di,dj]
    w_conv = np.asarray(w_conv, np.float64)
    oi = np.arange(26)
    oj = np.arange(26)
    wc = np.zeros((784, 676), np.float64)
    for di in range(3):
        for dj in range(3):
            src = ((oi[:, None] + di) * 28 + (oj[None, :] + dj)).ravel()
            dst = (oi[:, None] * 26 + oj[None, :]).ravel()
            wc[src, dst] += w_conv[di, dj]
    w_eff = (wc @ np.asarray(w1, np.float64)).astype(bf16)  # [784, 256]

    # wta[p, (m*NKT+t)*MT + j] = w_eff[t*KT+p, m*MT+j]  (m-major)
    wta = np.ascontiguousarray(
        w_eff.reshape(NKT, KT, NMT, MT).transpose(1, 2, 0, 3).reshape(KT, -1)
    )
    # b1a[p, m] = b1[m*MT+p]
    b1a = np.ascontiguousarray(
        np.asarray(b1, np.float32).reshape(NMT, MT).T
    )
    # w2a[p, m*OUT_PAD + j] = w2_padded[m*MT+p, j]
    w2p = np.zeros((CH, OUT_PAD), bf16)
    w2p[:, :OUT_CH] = np.asarray(w2).astype(bf16)
    w2a = np.ascontiguousarray(
        w2p.reshape(NMT, MT, OUT_PAD).transpose(1, 0, 2).reshape(MT, -1)
    )

    x_bf = np.asarray(x).astype(bf16)  # [B, 784]
    in_maps = []
    for c in range(N_CORES):
        shard = x_bf[c * b_shard:(c + 1) * b_shard]
        # xT[p, t, col] = shard[col, t*KT+p]
        xp = np.ascontiguousarray(
            shard.T.reshape(NKT, KT, b_shard).transpose(1, 0, 2)
        )
        in_maps.append({"xT": xp, "wta": wta, "b1a": b1a, "w2a": w2a})
    return in_maps


LAST_RESULT = None  # BassKernelResults of the most recent run (for test harness)


def kernel(x, w_conv, w1, b1, w2, b2):
    global LAST_RESULT
    nc = _get_nc()
    in_maps = _host_prep(x, w_conv, w1, b1, w2, b2)
    trace = bool(int(os.environ.get("KERNEL_TRACE", "0")))
    res = run_bass_kernel_spmd(
        nc, in_maps, list(range(N_CORES)), trace=trace,
        tmpdir=os.environ.get("KERNEL_TMPDIR") or None,
    )
    LAST_RESULT = res
    b2f = np.asarray(b2, np.float32)
    out = np.empty((B, OUT_CH), np.float32)
    for c in range(N_CORES):
        out[c * B_SHARD:(c + 1) * B_SHARD] = (
            res.results[c]["outT"][:OUT_CH].T + b2f
        )
    return out
